# revision 42
# baseline (speedup 1.0000x reference)
"""Trainium2 Bass kernel for an AttentionBlock (InstanceNorm + single-head
spatial self-attention + projection + residual).

Full-input contract: kernel(**inputs) takes the complete tensors and returns
the complete output. Internally shards across 8 NeuronCores: data-parallel
over batch (B=4 -> 4 pairs of cores), sequence-parallel over the N=4096 query
positions within each sample (2 cores per sample, 2048 queries each).

All 8 cores run the *same* program; the query-half assignment is done by
rotating the spatial columns of x host-side (attention and instance-norm
statistics are invariant under column permutation).

v9 design (two-phase projections):
  - startup critical path is DVE (stats -> 32 chunk-0 pass-1 row-max
    reduces). Projections run twice: a hi-only f16 pass feeds pass-1 (a max
    estimate tolerates f16-weight error) as soon as rstd exists, and the
    accurate hi+lo pass overwrites kst/qrhs later for pass-2 (the tile
    framework's subtile dependencies order the overwrite after the pass-1
    readers). fp8 casts (GpSimd) also read the hi-only versions - fp8 noise
    dwarfs the difference.
  - pass-1: fp8 DoubleRow matmuls (channel-split [32,2,*] operands, 2x PE
    throughput) for chunks 1-3, f16 for chunk 0; all ci-major so reduces
    start with the first k chunk. bf16 attention values absorb the fp8 max
    error; the per-row exp scale error cancels in the normalization.
  - pass-2: K=65 matmul pairs (64 channels + bias row carrying -rowmax);
    ScalarE exp writes bf16 attention straight to SBUF.
  - attn@v: hi-only f16 v projection (vst is bf16 anyway); ones column
    yields denominators.
  - epilogue: one K=65 matmul with wo1=[[WoT,0],[0,1...1]] gives projection
    + denominator broadcast; DVE reciprocal+mul, then one
    scalar_tensor_tensor adds bias and residual.
  - PE warm-up opens the HAM clock gate (k=8/8 grants follow dense
    activity).
"""

import os
import sys
import numpy as np
from contextlib import ExitStack

for _p in ("/opt/trn_rl_repo", "/root/.axon_site/_ro/trn_rl_repo"):
    if os.path.isdir(_p) and _p not in sys.path:
        sys.path.append(_p)

from concourse import bass, bacc, tile, mybir, masks  # noqa: E402
from concourse.bass_utils import run_bass_kernel_spmd  # noqa: E402

F32 = mybir.dt.float32
F16 = mybir.dt.float16
BF16 = mybir.dt.bfloat16
F8 = mybir.dt.float8e4

B, C, H, W = 4, 64, 64, 64
N = H * W            # 4096 spatial positions (attention length)
HALF = N // 2        # queries per core
KT = 128             # pass-2 k-tile (partition dim of transposed scores)
NKT = N // KT        # 32 k-tiles
NPR = NKT // 2       # 16 k-tile pairs
QC = 512             # q-chunk (PSUM bank free dim)
NQC = HALF // QC     # 4 q-chunks per core
QT = 128             # pass-1 q-tile
KC = 512             # pass-1 k-chunk
NKC = N // KC        # 8
EPS = 1e-5
NCORES = 8
USE_FP8_P1 = os.environ.get("USE_FP8_P1", "1") == "1"
WARM_N = int(os.environ.get("WARM_N", "26"))
AV_LAG = int(os.environ.get("AV_LAG", "3"))


def build_nc():
    nc = bacc.Bacc("TRN2", target_bir_lowering=False, debug=False)

    x_d = nc.dram_tensor("x", [C, N], F32, kind="ExternalInput")
    wq_d = nc.dram_tensor("wq1", [C + 1, 2, C], F16, kind="ExternalInput")
    wk_d = nc.dram_tensor("wk1", [C + 1, 2, C], F16, kind="ExternalInput")
    wv_d = nc.dram_tensor("wv1", [C + 1, 2, C], F16, kind="ExternalInput")
    wo_d = nc.dram_tensor("wo1", [C + 1, KT], BF16, kind="ExternalInput")
    bo_d = nc.dram_tensor("bo", [C, 1], F32, kind="ExternalInput")
    cs16_d = nc.dram_tensor("cst16", [2, N], F16, kind="ExternalInput")
    vini_d = nc.dram_tensor("vinit", [KT, NKT * 66], BF16, kind="ExternalInput")
    out_d = nc.dram_tensor("out", [C, HALF], F32, kind="ExternalOutput")

    with tile.TileContext(nc) as tc:
        _body(tc, x_d, wq_d, wk_d, wv_d, wo_d, bo_d, cs16_d, vini_d, out_d)
    nc.compile()
    return nc


def _body(tc, x_d, wq_d, wk_d, wv_d, wo_d, bo_d, cs16_d, vini_d, out_d):
    nc = tc.nc
    with ExitStack() as ctx:
        persist = ctx.enter_context(tc.tile_pool(name="persist", bufs=1))
        small = ctx.enter_context(tc.tile_pool(name="small", bufs=10))
        apool = ctx.enter_context(tc.tile_pool(name="apool", bufs=6))
        fpool = ctx.enter_context(tc.tile_pool(name="fpool", bufs=2))
        # PSUM budget (8 banks): p1p 2 + scp 4 + avp 2
        p1p = ctx.enter_context(tc.tile_pool(name="p1p", bufs=2, space="PSUM"))
        scp = ctx.enter_context(tc.tile_pool(name="scp", bufs=2, space="PSUM"))
        avp = ctx.enter_context(tc.tile_pool(name="avp", bufs=2, space="PSUM"))

        # ---- persistent tiles ----
        x_sb = persist.tile([C, N], F32)
        wq_sb = persist.tile([C + 1, 2, C], F16)
        wk_sb = persist.tile([C + 1, 2, C], F16)
        wv_sb = persist.tile([C + 1, 2, C], F16)
        wo_sb = persist.tile([C + 1, KT], BF16)
        bo_sb = persist.tile([C, 1], F32)
        xn = persist.tile([C, N], F32)
        xnh = persist.tile([C + 1, N], F16)
        xnl = persist.tile([C + 1, N], F16)
        kst = persist.tile([C + 1, N], F16)      # rows 0:64 k, row 64 ones
        qrhs = persist.tile([C + 1, HALF], F16)  # rows 0:64 q*sqrt(C), row 64 -max
        vst = persist.tile([KT, NKT, 66], BF16)  # [kpos, ktile, 64 v + ones + pad]
        k8 = persist.tile([C // 2, 2, N], F8)
        q8 = persist.tile([C // 2, 2, HALF], F8)
        ao_aug = persist.tile([C + 1, HALF], BF16)  # rows 0:64 attn@v, 64 denom
        stats = persist.tile([C, NKC, nc.vector.BN_STATS_DIM], F32)

        # ---- weight DMAs + constant rows via DMA ----
        nc.sync.dma_start(out=wq_sb, in_=wq_d.ap())
        nc.scalar.dma_start(out=wk_sb, in_=wk_d.ap())
        nc.sync.dma_start(out=wv_sb, in_=wv_d.ap())
        nc.scalar.dma_start(out=wo_sb, in_=wo_d.ap())
        nc.sync.dma_start(out=bo_sb, in_=bo_d.ap())
        nc.scalar.dma_start(out=xnh[C:C + 1, :], in_=cs16_d.ap()[0:1, :])
        nc.sync.dma_start(out=xnl[C:C + 1, :], in_=cs16_d.ap()[1:2, :])
        nc.scalar.dma_start(out=kst[C:C + 1, :], in_=cs16_d.ap()[0:1, :])
        # full-tile vst prefill (ones column + zero pad; contiguous per
        # partition, so 128 clean DMA descriptors - a strided column write
        # would be thousands of 2-byte descriptors)
        nc.gpsimd.dma_start(out=vst.rearrange("p a b -> p (a b)"),
                            in_=vini_d.ap())

        eps_t = persist.tile([C, 1], F32)
        nc.vector.memset(eps_t, EPS)
        dmy = persist.tile([QT, KC], F16)
        nc.gpsimd.memset(dmy, 0.25)
        ident = persist.tile([QT, QT], F16)
        masks.make_identity(nc, ident)

        def warm_pe(tag, n):
            """Back-to-back dummy matmuls (no readers -> no stalls) that keep
            the PE queue dense so the HAM clock gate opens to 8/8."""
            for w in range(n):
                wp = p1p.tile([QT, KC], F32, tag="p1", name=f"wp{tag}_{w}")
                nc.tensor.matmul(wp, lhsT=ident, rhs=dmy,
                                 start=True, stop=True, skip_group_check=True)

        # x load split across 3 DMA queues; bn_stats per chunk as it arrives
        warm_pe("h", WARM_N)
        dma_engs = [nc.sync, nc.scalar, nc.gpsimd]
        for i in range(NKC):
            sl = slice(i * KC, (i + 1) * KC)
            dma_engs[i % 3].dma_start(out=x_sb[:, sl], in_=x_d.ap()[:, sl])
            nc.vector.bn_stats(out=stats[:, i, :], in_=x_sb[:, sl])
        mv = persist.tile([C, nc.vector.BN_AGGR_DIM], F32)
        nc.vector.bn_aggr(out=mv, in_=stats)
        stdv = persist.tile([C, 1], F32)
        nc.scalar.activation(out=stdv, in_=mv[:, 1:2],
                             func=mybir.ActivationFunctionType.Sqrt,
                             bias=eps_t, scale=1.0)
        rstd = persist.tile([C, 1], F32)
        nc.vector.reciprocal(out=rstd, in_=stdv)
        nmr = persist.tile([C, 1], F32)
        nc.vector.tensor_mul(nmr, mv[:, 0:1], rstd)
        nc.vector.tensor_scalar_mul(nmr, nmr, -1.0)

        # ---- hi-only fast path: xnh -> kp_hi -> kst (row-max grade) ----
        for i in range(NKC):
            sl = slice(i * KC, (i + 1) * KC)
            nc.scalar.activation(out=xnh[0:C, sl], in_=x_sb[:, sl],
                                 func=mybir.ActivationFunctionType.Identity,
                                 bias=nmr, scale=rstd)
            kp = scp.tile([KT, 2, KC], F32, tag="sc", name=f"kp{i}")
            nc.tensor.matmul(kp[0:C, 0, :], lhsT=wk_sb[:, 0, :], rhs=xnh[:, sl],
                             start=True, stop=True, skip_group_check=True)
            nc.scalar.copy(kst[0:C, sl], kp[0:C, 0, :])
            # fp8 k casts from the hi-only k (fp8 noise dwarfs the lo term)
            nc.gpsimd.tensor_copy(k8[:, 0, sl], kst[0:C // 2, sl])
            nc.gpsimd.tensor_copy(k8[:, 1, sl], kst[C // 2:C, sl])
        for i in range(NQC):
            sl = slice(i * QC, (i + 1) * QC)
            qp = scp.tile([KT, 2, KC], F32, tag="sc", name=f"qp{i}")
            nc.tensor.matmul(qp[0:C, 0, :], lhsT=wq_sb[:, 0, :], rhs=xnh[:, sl],
                             start=True, stop=True, skip_group_check=True)
            nc.vector.tensor_copy(qrhs[0:C, sl], qp[0:C, 0, :])
            if i > 0:  # chunk 0 pass-1 runs f16 straight off qrhs
                nc.gpsimd.tensor_copy(q8[:, 0, sl], qrhs[0:C // 2, sl])
                nc.gpsimd.tensor_copy(q8[:, 1, sl], qrhs[C // 2:C, sl])

        # ---- incremental pass-1 (row max of chunk c1's q-tiles) ----
        # ci-major: one k-chunk matmul + DVE row-max per step; a q-tile's
        # -max lands in qrhs row 64 via a PE transpose after its last chunk.
        p1_state = {}

        def pass1_step(c1):
            st = p1_state.setdefault(c1, {"step": 0, "cm": {}})
            step = st["step"]
            if step >= 4 * NKC:
                return
            st["step"] = step + 1
            ci, t4 = divmod(step, 4)
            t = c1 * 4 + t4
            tq = slice(t * QT, (t + 1) * QT)
            if ci == 0:
                st["cm"][t4] = small.tile([QT, NKC], F32, tag="cm",
                                          name=f"cm{t}")
            cm = st["cm"][t4]
            cs = slice(ci * KC, (ci + 1) * KC)
            p1 = p1p.tile([QT, KC], F32, tag="p1", name=f"p1_{t}_{ci}")
            if USE_FP8_P1 and c1 > 0:
                nc.tensor.matmul(p1, lhsT=q8[:, :, tq], rhs=k8[:, :, cs],
                                 perf_mode=mybir.MatmulPerfMode.DoubleRow,
                                 start=True, stop=True, skip_group_check=True)
            else:
                nc.tensor.matmul(p1, lhsT=qrhs[0:C, tq], rhs=kst[0:C, cs],
                                 start=True, stop=True, skip_group_check=True)
            nc.vector.tensor_reduce(cm[:, ci:ci + 1], p1,
                                    axis=mybir.AxisListType.X,
                                    op=mybir.AluOpType.max)
            if ci == NKC - 1:
                nmT = small.tile([QT, 1], F16, tag="nmT", name=f"nmT{t}")
                nc.vector.tensor_reduce(nmT, cm,
                                        axis=mybir.AxisListType.X,
                                        op=mybir.AluOpType.max, negate=True)
                tr = p1p.tile([1, QT], F16, tag="p1", name=f"tr{t}")
                nc.tensor.transpose(tr, nmT, ident)
                nc.scalar.copy(qrhs[C:C + 1, tq], tr[0:1, :])

        # ---- v projection (hi-only) + chunk-0 pass-1 + lo corrections ----
        # xn/xnl (GpSimd) trickle in alongside; the accurate hi+lo k/q
        # projections overwrite kst/qrhs before the conveyor reads them
        # (subtile deps order the overwrite after pass-1/fp8-cast readers).
        def full_kq(i):
            if i < NKC:
                sl = slice(i * KC, (i + 1) * KC)
                kp = scp.tile([KT, 2, KC], F32, tag="sc", name=f"kpf{i}")
                nc.tensor.matmul(kp[0:C, 0, :], lhsT=wk_sb[:, 0, :],
                                 rhs=xnh[:, sl],
                                 start=True, stop=False, skip_group_check=True)
                nc.tensor.matmul(kp[0:C, 0, :], lhsT=wk_sb[:, 0, :],
                                 rhs=xnl[:, sl],
                                 start=False, stop=True, skip_group_check=True)
                nc.scalar.copy(kst[0:C, sl], kp[0:C, 0, :])
            else:
                q = i - NKC
                sl = slice(q * QC, (q + 1) * QC)
                qp = scp.tile([KT, 2, KC], F32, tag="sc", name=f"qpf{q}")
                nc.tensor.matmul(qp[0:C, 0, :], lhsT=wq_sb[:, 0, :],
                                 rhs=xnh[:, sl],
                                 start=True, stop=False, skip_group_check=True)
                nc.tensor.matmul(qp[0:C, 0, :], lhsT=wq_sb[:, 0, :],
                                 rhs=xnl[:, sl],
                                 start=False, stop=True, skip_group_check=True)
                nc.vector.tensor_copy(qrhs[0:C, sl], qp[0:C, 0, :])

        for i in range(NKC):
            sl = slice(i * KC, (i + 1) * KC)
            nc.gpsimd.tensor_scalar(out=xn[:, sl], in0=x_sb[:, sl],
                                    scalar1=rstd, scalar2=nmr,
                                    op0=mybir.AluOpType.mult,
                                    op1=mybir.AluOpType.add)
            nc.gpsimd.tensor_sub(xnl[0:C, sl], xn[:, sl], xnh[0:C, sl])

        for j in range(NKT):
            js = slice(j * KT, (j + 1) * KT)
            vp = p1p.tile([KT, C], F32, tag="p1", name=f"vp{j}")
            nc.tensor.matmul(vp, lhsT=xnh[:, js], rhs=wv_sb[:, 0, :],
                             start=True, stop=True, skip_group_check=True)
            nc.scalar.copy(vst[:, j, 0:C], vp)
            pass1_step(0)
            if j >= 8 and j % 2 == 0:
                full_kq((j - 8) // 2)  # covers all 12 (8 k + 4 q) by j=30

        # ---- main loop over q-chunks ----
        # attn@v matmuls trail the score/exp conveyor by AV_LAG pairs so they
        # never head-of-line block the in-order PE queue on a fresh exp.
        av_fifo = []

        def emit_av(c, p, otp, ab):
            for h in range(2):
                j = 2 * p + h
                nc.tensor.matmul(otp, lhsT=vst[:, j, :], rhs=ab[:, h, :],
                                 start=(j == 0), stop=(j == NKT - 1),
                                 skip_group_check=True)

        def emit_epilogue(c, otp):
            qs = slice(c * QC, (c + 1) * QC)
            # single copy moves attn@v rows AND the denominator row; DVE
            # keeps it off the exp-laden ScalarE queue so the fx matmul
            # doesn't head-of-line block the PE behind pending exps.
            nc.vector.tensor_copy(ao_aug[:, qs], otp[0:C + 1, :])
            # fx rows 0:64 = Wo @ attn@v ; rows 64:128 = denominator bcast
            fx = p1p.tile([KT, QC], F32, tag="p1", name=f"fx{c}")
            nc.tensor.matmul(fx, lhsT=wo_sb, rhs=ao_aug[:, qs],
                             start=True, stop=True, skip_group_check=True)
            # (DVE has no divide ALU op and reads only one PSUM operand.)
            ibs = fpool.tile([C, QC], F32, tag="ibs", name=f"ib{c}")
            nc.vector.reciprocal(out=ibs, in_=fx[C:2 * C, :])
            fin = fpool.tile([C, QC], F32, tag="fin", name=f"fin{c}")
            nc.vector.tensor_mul(fin, fx[0:C, :], ibs)
            nc.vector.scalar_tensor_tensor(out=fin, in0=fin, scalar=bo_sb,
                                           in1=xn[:, qs],
                                           op0=mybir.AluOpType.add,
                                           op1=mybir.AluOpType.add)
            eng = nc.sync if c % 2 == 0 else nc.scalar
            eng.dma_start(out=out_d.ap()[:, qs], in_=fin)

        def pop_av():
            c0, p0, otp0, ab0 = av_fifo.pop(0)
            emit_av(c0, p0, otp0, ab0)
            if p0 == NPR - 1:
                emit_epilogue(c0, otp0)

        for c in range(NQC):
            qs = slice(c * QC, (c + 1) * QC)
            otp = avp.tile([66, QC], F32, tag="av", name=f"otp{c}")
            for p in range(NPR):
                sc = scp.tile([KT, 2, QC], F32, tag="sc", name=f"sc{c}_{p}")
                for h in range(2):
                    js = slice((2 * p + h) * KT, (2 * p + h + 1) * KT)
                    nc.tensor.matmul(sc[:, h, :], lhsT=kst[:, js],
                                     rhs=qrhs[:, qs],
                                     start=True, stop=True,
                                     skip_group_check=True)
                ab = apool.tile([KT, 2, QC], BF16, tag="ab", name=f"ab{c}_{p}")
                nc.scalar.activation(out=ab.rearrange("p a b -> p (a b)"),
                                     in_=sc.rearrange("p a b -> p (a b)"),
                                     func=mybir.ActivationFunctionType.Exp,
                                     bias=0.0, scale=1.0)
                av_fifo.append((c, p, otp, ab))
                if len(av_fifo) > AV_LAG:
                    pop_av()
                if c + 1 < NQC:
                    # 3 steps/pair drains the 32 steps by pair 11 so the
                    # next chunk's -max bias lands well before the boundary.
                    pass1_step(c + 1)
                    pass1_step(c + 1)
                    pass1_step(c + 1)
        while av_fifo:
            pop_av()


def prep_inputs(x, w_qkv, b_qkv, w_out, b_out):
    """Host-side slicing/packing into per-core input maps."""
    import ml_dtypes
    x = np.asarray(x, dtype=np.float32).reshape(B, C, N)
    w_qkv = np.asarray(w_qkv, dtype=np.float32)
    b_qkv = np.asarray(b_qkv, dtype=np.float32)
    w_out = np.asarray(w_out, dtype=np.float32)
    b_out = np.asarray(b_out, dtype=np.float32)

    s = float(C) ** 0.5  # reference multiplies scores by sqrt(C)
    wq1 = np.concatenate([s * w_qkv[0:C].T, s * b_qkv[None, 0:C]], axis=0)
    wk1 = np.concatenate([w_qkv[C:2 * C].T, b_qkv[None, C:2 * C]], axis=0)
    wv1 = np.concatenate([w_qkv[2 * C:3 * C].T, b_qkv[None, 2 * C:3 * C]], axis=0)

    def hilo16(w):  # [65, 64] -> [65, 2, 64] f16 (hi, lo), hi+lo ~== w
        hi = w.astype(np.float16)
        lo = (w - hi.astype(np.float32)).astype(np.float16)
        return np.ascontiguousarray(np.stack([hi, lo], axis=1))

    wq1 = hilo16(np.ascontiguousarray(wq1))
    wk1 = hilo16(np.ascontiguousarray(wk1))
    wv1 = hilo16(np.ascontiguousarray(wv1))
    # wo1: [65, 128]; rows 0:64 cols 0:64 = WoT; row 64 cols 64:128 = 1
    # so one K=65 matmul gives [Wo@ao ; denom broadcast] stacked.
    wo1 = np.zeros((C + 1, KT), dtype=np.float32)
    wo1[0:C, 0:C] = w_out.T
    wo1[C, C:KT] = 1.0
    wo1 = np.ascontiguousarray(wo1).astype(ml_dtypes.bfloat16)
    bo = np.ascontiguousarray(b_out[:, None])
    cst16 = np.zeros((2, N), dtype=np.float16)
    cst16[0] = 1.0
    # vst prefill: zeros with ones in column 64 of each [128, 66] k-tile
    vinit = np.zeros((KT, NKT, 66), dtype=ml_dtypes.bfloat16)
    vinit[:, :, 64] = 1.0
    vinit = np.ascontiguousarray(vinit.reshape(KT, NKT * 66))

    in_maps = []
    for j in range(NCORES):
        b, h = divmod(j, 2)
        xs = x[b]
        if h == 1:
            xs = np.concatenate([xs[:, HALF:], xs[:, :HALF]], axis=1)
        in_maps.append({
            "x": np.ascontiguousarray(xs),
            "wq1": wq1,
            "wk1": wk1,
            "wv1": wv1,
            "wo1": wo1,
            "bo": bo,
            "cst16": cst16,
            "vinit": vinit,
        })
    return in_maps


def gather_output(results):
    out = np.empty((B, C, N), dtype=np.float32)
    for j in range(NCORES):
        b, h = divmod(j, 2)
        out[b][:, h * HALF:(h + 1) * HALF] = results[j]["out"]
    return out.reshape(B, C, H, W)


_NC_CACHE = {}


def get_nc():
    key = "v9"
    if key not in _NC_CACHE:
        _NC_CACHE[key] = build_nc()
    return _NC_CACHE[key]


def kernel(x, w_qkv, b_qkv, w_out, b_out):
    nc = get_nc()
    in_maps = prep_inputs(x, w_qkv, b_qkv, w_out, b_out)
    res = run_bass_kernel_spmd(nc, in_maps, list(range(NCORES)))
    return gather_output(res.results)


# revision 43
# speedup vs baseline: 1.1033x; 1.1033x over previous
"""Trainium2 Bass kernel for an AttentionBlock (InstanceNorm + single-head
spatial self-attention + projection + residual).

Full-input contract: kernel(**inputs) takes the complete tensors and returns
the complete output. Internally shards across 8 NeuronCores: data-parallel
over batch (B=4 -> 4 pairs of cores), sequence-parallel over the N=4096 query
positions within each sample (2 cores per sample, 2048 queries each).

All 8 cores run the *same* program; the query-half assignment is done by
rotating the spatial columns of x host-side (attention and instance-norm
statistics are invariant under column permutation).

v9 design (two-phase projections):
  - startup critical path is DVE (stats -> 32 chunk-0 pass-1 row-max
    reduces). Projections run twice: a hi-only f16 pass feeds pass-1 (a max
    estimate tolerates f16-weight error) as soon as rstd exists, and the
    accurate hi+lo pass overwrites kst/qrhs later for pass-2 (the tile
    framework's subtile dependencies order the overwrite after the pass-1
    readers). fp8 casts (GpSimd) also read the hi-only versions - fp8 noise
    dwarfs the difference.
  - pass-1: fp8 DoubleRow matmuls (channel-split [32,2,*] operands, 2x PE
    throughput) for chunks 1-3, f16 for chunk 0; all ci-major so reduces
    start with the first k chunk. bf16 attention values absorb the fp8 max
    error; the per-row exp scale error cancels in the normalization.
  - pass-2: K=65 matmul pairs (64 channels + bias row carrying -rowmax);
    ScalarE exp writes bf16 attention straight to SBUF.
  - attn@v: hi-only f16 v projection (vst is bf16 anyway); ones column
    yields denominators.
  - epilogue: one K=65 matmul with wo1=[[WoT,0],[0,1...1]] gives projection
    + denominator broadcast; DVE reciprocal+mul, then one
    scalar_tensor_tensor adds bias and residual.
  - PE warm-up opens the HAM clock gate (k=8/8 grants follow dense
    activity).
"""

import os
import sys
import numpy as np
from contextlib import ExitStack

for _p in ("/opt/trn_rl_repo", "/root/.axon_site/_ro/trn_rl_repo"):
    if os.path.isdir(_p) and _p not in sys.path:
        sys.path.append(_p)

from concourse import bass, bacc, tile, mybir, masks  # noqa: E402
from concourse.bass_utils import run_bass_kernel_spmd  # noqa: E402

F32 = mybir.dt.float32
F16 = mybir.dt.float16
BF16 = mybir.dt.bfloat16
F8 = mybir.dt.float8e4

B, C, H, W = 4, 64, 64, 64
N = H * W            # 4096 spatial positions (attention length)
HALF = N // 2        # queries per core
KT = 128             # pass-2 k-tile (partition dim of transposed scores)
NKT = N // KT        # 32 k-tiles
NPR = NKT // 2       # 16 k-tile pairs
QC = 512             # q-chunk (PSUM bank free dim)
NQC = HALF // QC     # 4 q-chunks per core
QT = 128             # pass-1 q-tile
KC = 512             # pass-1 k-chunk
NKC = N // KC        # 8
EPS = 1e-5
NCORES = 8
USE_FP8_P1 = os.environ.get("USE_FP8_P1", "1") == "1"
WARM_N = int(os.environ.get("WARM_N", "26"))
AV_LAG = int(os.environ.get("AV_LAG", "3"))


def build_nc():
    nc = bacc.Bacc("TRN2", target_bir_lowering=False, debug=False)

    x_d = nc.dram_tensor("x", [C, N], F32, kind="ExternalInput")
    wq_d = nc.dram_tensor("wq1", [C + 1, 2, C], F16, kind="ExternalInput")
    wk_d = nc.dram_tensor("wk1", [C + 1, 2, C], F16, kind="ExternalInput")
    wv_d = nc.dram_tensor("wv1", [C + 1, 2, C], F16, kind="ExternalInput")
    wo_d = nc.dram_tensor("wo1", [C + 1, KT], BF16, kind="ExternalInput")
    bo_d = nc.dram_tensor("bo", [C, 1], F32, kind="ExternalInput")
    cs16_d = nc.dram_tensor("cst16", [2, N], F16, kind="ExternalInput")
    vini_d = nc.dram_tensor("vinit", [KT, NKT * 66], BF16, kind="ExternalInput")
    out_d = nc.dram_tensor("out", [C, HALF], F32, kind="ExternalOutput")

    with tile.TileContext(nc) as tc:
        _body(tc, x_d, wq_d, wk_d, wv_d, wo_d, bo_d, cs16_d, vini_d, out_d)
    nc.compile()
    return nc


def _body(tc, x_d, wq_d, wk_d, wv_d, wo_d, bo_d, cs16_d, vini_d, out_d):
    nc = tc.nc
    with ExitStack() as ctx:
        persist = ctx.enter_context(tc.tile_pool(name="persist", bufs=1))
        small = ctx.enter_context(tc.tile_pool(name="small", bufs=10))
        apool = ctx.enter_context(tc.tile_pool(name="apool", bufs=6))
        fpool = ctx.enter_context(tc.tile_pool(name="fpool", bufs=2))
        # PSUM budget (8 banks): p1p 2 + scp 4 + avp 2
        p1p = ctx.enter_context(tc.tile_pool(name="p1p", bufs=2, space="PSUM"))
        scp = ctx.enter_context(tc.tile_pool(name="scp", bufs=2, space="PSUM"))
        avp = ctx.enter_context(tc.tile_pool(name="avp", bufs=2, space="PSUM"))

        # ---- persistent tiles ----
        x_sb = persist.tile([C, N], F32)
        wq_sb = persist.tile([C + 1, 2, C], F16)
        wk_sb = persist.tile([C + 1, 2, C], F16)
        wv_sb = persist.tile([C + 1, 2, C], F16)
        wo_sb = persist.tile([C + 1, KT], BF16)
        bo_sb = persist.tile([C, 1], F32)
        xn = persist.tile([C, N], F32)
        xnh = persist.tile([C + 1, N], F16)
        kst = persist.tile([C + 1, N], F16)      # rows 0:64 k, row 64 ones
        qrhs = persist.tile([C + 1, HALF], F16)  # rows 0:64 q*sqrt(C), row 64 -max
        vst = persist.tile([KT, NKT, 66], BF16)  # [kpos, ktile, 64 v + ones + pad]
        k8 = persist.tile([C // 2, 2, N], F8)
        q8 = persist.tile([C // 2, 2, HALF], F8)
        ao_aug = persist.tile([C + 1, HALF], BF16)  # rows 0:64 attn@v, 64 denom
        stats = persist.tile([C, NKC, nc.vector.BN_STATS_DIM], F32)

        # ---- weight DMAs + constant rows via DMA ----
        nc.sync.dma_start(out=wq_sb, in_=wq_d.ap())
        nc.scalar.dma_start(out=wk_sb, in_=wk_d.ap())
        nc.sync.dma_start(out=wv_sb, in_=wv_d.ap())
        nc.scalar.dma_start(out=wo_sb, in_=wo_d.ap())
        nc.sync.dma_start(out=bo_sb, in_=bo_d.ap())
        nc.scalar.dma_start(out=xnh[C:C + 1, :], in_=cs16_d.ap()[0:1, :])
        nc.scalar.dma_start(out=kst[C:C + 1, :], in_=cs16_d.ap()[0:1, :])
        # full-tile vst prefill (ones column + zero pad; contiguous per
        # partition, so 128 clean DMA descriptors - a strided column write
        # would be thousands of 2-byte descriptors)
        nc.gpsimd.dma_start(out=vst.rearrange("p a b -> p (a b)"),
                            in_=vini_d.ap())

        eps_t = persist.tile([C, 1], F32)
        nc.vector.memset(eps_t, EPS)
        dmy = persist.tile([QT, KC], F16)
        nc.gpsimd.memset(dmy, 0.25)
        ident = persist.tile([QT, QT], F16)
        masks.make_identity(nc, ident)

        def warm_pe(tag, n):
            """Back-to-back dummy matmuls (no readers -> no stalls) that keep
            the PE queue dense so the HAM clock gate opens to 8/8."""
            for w in range(n):
                wp = p1p.tile([QT, KC], F32, tag="p1", name=f"wp{tag}_{w}")
                nc.tensor.matmul(wp, lhsT=ident, rhs=dmy,
                                 start=True, stop=True, skip_group_check=True)

        # x load split across 3 DMA queues; bn_stats per chunk as it arrives
        warm_pe("h", WARM_N)
        dma_engs = [nc.sync, nc.scalar, nc.gpsimd]
        for i in range(NKC):
            sl = slice(i * KC, (i + 1) * KC)
            dma_engs[i % 3].dma_start(out=x_sb[:, sl], in_=x_d.ap()[:, sl])
            nc.vector.bn_stats(out=stats[:, i, :], in_=x_sb[:, sl])
        mv = persist.tile([C, nc.vector.BN_AGGR_DIM], F32)
        nc.vector.bn_aggr(out=mv, in_=stats)
        stdv = persist.tile([C, 1], F32)
        nc.scalar.activation(out=stdv, in_=mv[:, 1:2],
                             func=mybir.ActivationFunctionType.Sqrt,
                             bias=eps_t, scale=1.0)
        rstd = persist.tile([C, 1], F32)
        nc.vector.reciprocal(out=rstd, in_=stdv)
        nmr = persist.tile([C, 1], F32)
        nc.vector.tensor_mul(nmr, mv[:, 0:1], rstd)
        nc.vector.tensor_scalar_mul(nmr, nmr, -1.0)

        # ---- projections (single f16 pass: kst/qrhs storage is f16 and
        # the 2e-2 tolerance absorbs the f16-weight rounding) ----
        for i in range(NKC):
            sl = slice(i * KC, (i + 1) * KC)
            nc.scalar.activation(out=xnh[0:C, sl], in_=x_sb[:, sl],
                                 func=mybir.ActivationFunctionType.Identity,
                                 bias=nmr, scale=rstd)
            kp = scp.tile([KT, 2, KC], F32, tag="sc", name=f"kp{i}")
            nc.tensor.matmul(kp[0:C, 0, :], lhsT=wk_sb[:, 0, :], rhs=xnh[:, sl],
                             start=True, stop=True, skip_group_check=True)
            nc.scalar.copy(kst[0:C, sl], kp[0:C, 0, :])
            # fp8 k casts from the hi-only k (fp8 noise dwarfs the lo term)
            nc.gpsimd.tensor_copy(k8[:, 0, sl], kst[0:C // 2, sl])
            nc.gpsimd.tensor_copy(k8[:, 1, sl], kst[C // 2:C, sl])
        for i in range(NQC):
            sl = slice(i * QC, (i + 1) * QC)
            qp = scp.tile([KT, 2, KC], F32, tag="sc", name=f"qp{i}")
            nc.tensor.matmul(qp[0:C, 0, :], lhsT=wq_sb[:, 0, :], rhs=xnh[:, sl],
                             start=True, stop=True, skip_group_check=True)
            nc.vector.tensor_copy(qrhs[0:C, sl], qp[0:C, 0, :])
            if i > 0:  # chunk 0 pass-1 runs f16 straight off qrhs
                nc.gpsimd.tensor_copy(q8[:, 0, sl], qrhs[0:C // 2, sl])
                nc.gpsimd.tensor_copy(q8[:, 1, sl], qrhs[C // 2:C, sl])

        # ---- incremental pass-1 (row max of chunk c1's q-tiles) ----
        # ci-major: one k-chunk matmul + DVE row-max per step; a q-tile's
        # -max lands in qrhs row 64 via a PE transpose after its last chunk.
        p1_state = {}

        def pass1_step(c1):
            st = p1_state.setdefault(c1, {"step": 0, "cm": {}})
            step = st["step"]
            if step >= 4 * NKC:
                return
            st["step"] = step + 1
            ci, t4 = divmod(step, 4)
            t = c1 * 4 + t4
            tq = slice(t * QT, (t + 1) * QT)
            if ci == 0:
                st["cm"][t4] = small.tile([QT, NKC], F32, tag="cm",
                                          name=f"cm{t}")
            cm = st["cm"][t4]
            cs = slice(ci * KC, (ci + 1) * KC)
            p1 = p1p.tile([QT, KC], F32, tag="p1", name=f"p1_{t}_{ci}")
            if USE_FP8_P1 and c1 > 0:
                nc.tensor.matmul(p1, lhsT=q8[:, :, tq], rhs=k8[:, :, cs],
                                 perf_mode=mybir.MatmulPerfMode.DoubleRow,
                                 start=True, stop=True, skip_group_check=True)
            else:
                nc.tensor.matmul(p1, lhsT=qrhs[0:C, tq], rhs=kst[0:C, cs],
                                 start=True, stop=True, skip_group_check=True)
            nc.vector.tensor_reduce(cm[:, ci:ci + 1], p1,
                                    axis=mybir.AxisListType.X,
                                    op=mybir.AluOpType.max)
            if ci == NKC - 1:
                nmT = small.tile([QT, 1], F16, tag="nmT", name=f"nmT{t}")
                nc.vector.tensor_reduce(nmT, cm,
                                        axis=mybir.AxisListType.X,
                                        op=mybir.AluOpType.max, negate=True)
                tr = p1p.tile([1, QT], F16, tag="p1", name=f"tr{t}")
                nc.tensor.transpose(tr, nmT, ident)
                nc.scalar.copy(qrhs[C:C + 1, tq], tr[0:1, :])

        # ---- v projection + chunk-0 pass-1; xn (residual) on GpSimd ----
        for i in range(NKC):
            sl = slice(i * KC, (i + 1) * KC)
            nc.gpsimd.tensor_scalar(out=xn[:, sl], in0=x_sb[:, sl],
                                    scalar1=rstd, scalar2=nmr,
                                    op0=mybir.AluOpType.mult,
                                    op1=mybir.AluOpType.add)

        for j in range(NKT):
            js = slice(j * KT, (j + 1) * KT)
            vp = p1p.tile([KT, C], F32, tag="p1", name=f"vp{j}")
            nc.tensor.matmul(vp, lhsT=xnh[:, js], rhs=wv_sb[:, 0, :],
                             start=True, stop=True, skip_group_check=True)
            nc.scalar.copy(vst[:, j, 0:C], vp)
            pass1_step(0)

        # ---- main loop over q-chunks ----
        # attn@v matmuls trail the score/exp conveyor by AV_LAG pairs so they
        # never head-of-line block the in-order PE queue on a fresh exp.
        av_fifo = []

        def emit_av(c, p, otp, ab):
            for h in range(2):
                j = 2 * p + h
                nc.tensor.matmul(otp, lhsT=vst[:, j, :], rhs=ab[:, h, :],
                                 start=(j == 0), stop=(j == NKT - 1),
                                 skip_group_check=True)

        def emit_epilogue(c, otp):
            qs = slice(c * QC, (c + 1) * QC)
            # single copy moves attn@v rows AND the denominator row; DVE
            # keeps it off the exp-laden ScalarE queue so the fx matmul
            # doesn't head-of-line block the PE behind pending exps.
            nc.vector.tensor_copy(ao_aug[:, qs], otp[0:C + 1, :])
            # fx rows 0:64 = Wo @ attn@v ; rows 64:128 = denominator bcast
            fx = p1p.tile([KT, QC], F32, tag="p1", name=f"fx{c}")
            nc.tensor.matmul(fx, lhsT=wo_sb, rhs=ao_aug[:, qs],
                             start=True, stop=True, skip_group_check=True)
            # (DVE has no divide ALU op and reads only one PSUM operand.)
            ibs = fpool.tile([C, QC], F32, tag="ibs", name=f"ib{c}")
            nc.vector.reciprocal(out=ibs, in_=fx[C:2 * C, :])
            fin = fpool.tile([C, QC], F32, tag="fin", name=f"fin{c}")
            nc.vector.tensor_mul(fin, fx[0:C, :], ibs)
            nc.vector.scalar_tensor_tensor(out=fin, in0=fin, scalar=bo_sb,
                                           in1=xn[:, qs],
                                           op0=mybir.AluOpType.add,
                                           op1=mybir.AluOpType.add)
            eng = nc.sync if c % 2 == 0 else nc.scalar
            eng.dma_start(out=out_d.ap()[:, qs], in_=fin)

        def pop_av():
            c0, p0, otp0, ab0 = av_fifo.pop(0)
            emit_av(c0, p0, otp0, ab0)
            if p0 == NPR - 1:
                emit_epilogue(c0, otp0)

        for c in range(NQC):
            qs = slice(c * QC, (c + 1) * QC)
            otp = avp.tile([66, QC], F32, tag="av", name=f"otp{c}")
            for p in range(NPR):
                sc = scp.tile([KT, 2, QC], F32, tag="sc", name=f"sc{c}_{p}")
                for h in range(2):
                    js = slice((2 * p + h) * KT, (2 * p + h + 1) * KT)
                    nc.tensor.matmul(sc[:, h, :], lhsT=kst[:, js],
                                     rhs=qrhs[:, qs],
                                     start=True, stop=True,
                                     skip_group_check=True)
                ab = apool.tile([KT, 2, QC], BF16, tag="ab", name=f"ab{c}_{p}")
                nc.scalar.activation(out=ab.rearrange("p a b -> p (a b)"),
                                     in_=sc.rearrange("p a b -> p (a b)"),
                                     func=mybir.ActivationFunctionType.Exp,
                                     bias=0.0, scale=1.0)
                av_fifo.append((c, p, otp, ab))
                if len(av_fifo) > AV_LAG:
                    pop_av()
                if c + 1 < NQC:
                    # 3 steps/pair drains the 32 steps by pair 11 so the
                    # next chunk's -max bias lands well before the boundary.
                    pass1_step(c + 1)
                    pass1_step(c + 1)
                    pass1_step(c + 1)
        while av_fifo:
            pop_av()


def prep_inputs(x, w_qkv, b_qkv, w_out, b_out):
    """Host-side slicing/packing into per-core input maps."""
    import ml_dtypes
    x = np.asarray(x, dtype=np.float32).reshape(B, C, N)
    w_qkv = np.asarray(w_qkv, dtype=np.float32)
    b_qkv = np.asarray(b_qkv, dtype=np.float32)
    w_out = np.asarray(w_out, dtype=np.float32)
    b_out = np.asarray(b_out, dtype=np.float32)

    s = float(C) ** 0.5  # reference multiplies scores by sqrt(C)
    wq1 = np.concatenate([s * w_qkv[0:C].T, s * b_qkv[None, 0:C]], axis=0)
    wk1 = np.concatenate([w_qkv[C:2 * C].T, b_qkv[None, C:2 * C]], axis=0)
    wv1 = np.concatenate([w_qkv[2 * C:3 * C].T, b_qkv[None, 2 * C:3 * C]], axis=0)

    def hilo16(w):  # [65, 64] -> [65, 2, 64] f16 (hi, lo), hi+lo ~== w
        hi = w.astype(np.float16)
        lo = (w - hi.astype(np.float32)).astype(np.float16)
        return np.ascontiguousarray(np.stack([hi, lo], axis=1))

    wq1 = hilo16(np.ascontiguousarray(wq1))
    wk1 = hilo16(np.ascontiguousarray(wk1))
    wv1 = hilo16(np.ascontiguousarray(wv1))
    # wo1: [65, 128]; rows 0:64 cols 0:64 = WoT; row 64 cols 64:128 = 1
    # so one K=65 matmul gives [Wo@ao ; denom broadcast] stacked.
    wo1 = np.zeros((C + 1, KT), dtype=np.float32)
    wo1[0:C, 0:C] = w_out.T
    wo1[C, C:KT] = 1.0
    wo1 = np.ascontiguousarray(wo1).astype(ml_dtypes.bfloat16)
    bo = np.ascontiguousarray(b_out[:, None])
    cst16 = np.zeros((2, N), dtype=np.float16)
    cst16[0] = 1.0
    # vst prefill: zeros with ones in column 64 of each [128, 66] k-tile
    vinit = np.zeros((KT, NKT, 66), dtype=ml_dtypes.bfloat16)
    vinit[:, :, 64] = 1.0
    vinit = np.ascontiguousarray(vinit.reshape(KT, NKT * 66))

    in_maps = []
    for j in range(NCORES):
        b, h = divmod(j, 2)
        xs = x[b]
        if h == 1:
            xs = np.concatenate([xs[:, HALF:], xs[:, :HALF]], axis=1)
        in_maps.append({
            "x": np.ascontiguousarray(xs),
            "wq1": wq1,
            "wk1": wk1,
            "wv1": wv1,
            "wo1": wo1,
            "bo": bo,
            "cst16": cst16,
            "vinit": vinit,
        })
    return in_maps


def gather_output(results):
    out = np.empty((B, C, N), dtype=np.float32)
    for j in range(NCORES):
        b, h = divmod(j, 2)
        out[b][:, h * HALF:(h + 1) * HALF] = results[j]["out"]
    return out.reshape(B, C, H, W)


_NC_CACHE = {}


def get_nc():
    key = "v10"
    if key not in _NC_CACHE:
        _NC_CACHE[key] = build_nc()
    return _NC_CACHE[key]


def kernel(x, w_qkv, b_qkv, w_out, b_out):
    nc = get_nc()
    in_maps = prep_inputs(x, w_qkv, b_qkv, w_out, b_out)
    res = run_bass_kernel_spmd(nc, in_maps, list(range(NCORES)))
    return gather_output(res.results)
